# revision 4
# baseline (speedup 1.0000x reference)
"""TRN2 Bass kernel for nn_BlockPermProduct.

The reference applies 9 probabilistic block-permutation mixing steps to each
row of x [65536, 1024]. Every step is linear in x, so the whole transform is
``out = x @ M^T`` for a 1024x1024 matrix M built on the host (float64) from
the tiny (9, 3) logits. Structure analysis (block sparsity, hierarchical
ranks, Monarch tests) shows M sits exactly at the low-rank break-even point
at every scale, so a dense matmul is the right device algorithm; the wins
come from engineering it to the PE streaming floor:

  - bf16 end to end: x is pre-transposed AND cast to bf16 on the host, so
    the device does ZERO transposes (the old kernel burned a third of PE
    time on them); HBM traffic halves; FWL makes weight loads 2x faster.
  - out^T formulation: psum[i, r] = sum_j MT[j_block, i_block]^T @ xT[j, r]
    with the 64 constant 128x128 MT sub-blocks as stationary weights and
    xT chunks as N=512 moving operands. Weight loads are data-independent
    and hide under the 512-cycle streams.
  - i-outer / j-inner loop: each output chunk's PSUM bank drains (Vector
    and Scalar engines alternate on the fp32->bf16 copies) while the next
    chunk's matmuls stream -> no PE bubbles.
  - per-group 1 MiB DMAs (in and out), triple-buffered.

Host un-transposes and casts the bf16 out^T back to fp32. Measured rel err
~1e-3 (tolerance 2e-2). Sharding: pure data parallel over batch, 8 cores.
"""

import numpy as np
import ml_dtypes
from contextlib import ExitStack

import concourse.bass as bass
import concourse.bacc as bacc
import concourse.mybir as mybir
import concourse.tile as tile
from concourse.bass_utils import run_bass_kernel_spmd

BATCH = 65536
SIZE = 1024
N_CORES = 8
ROWS_PER_CORE = BATCH // N_CORES  # 8192
P = 128
N_CHUNK = SIZE // P  # 8
GW = 512  # rows ("r" columns of xT) per group
N_GROUPS = ROWS_PER_CORE // GW  # 16
HALF = 512

F32 = mybir.dt.float32
BF16 = mybir.dt.bfloat16
NP_BF16 = np.dtype(ml_dtypes.bfloat16)

MATMUL_MODE = "bf16_xt"

TRACE = False
TRACE_KWARGS = {}
LAST_RESULTS = None

_NC_CACHE = {}


def _transform64(y, logits):
    """Float64 port of the reference transform, applied to rows of y."""
    m = 10
    sizes = [SIZE >> i for i in range(m - 1)][::-1]  # [4, 8, ..., 1024]
    out = y
    for i in range(m - 2, -1, -1):
        n = sizes[i]
        p = 1.0 / (1.0 + np.exp(-logits[i].astype(np.float64)))
        z = out.reshape(-1, n)
        sep = z.reshape(-1, n // 2, 2).transpose(0, 2, 1).reshape(-1, n)
        z = (1 - p[0]) * z + p[0] * sep
        h = n // 2
        first = (1 - p[1]) * z[:, :h] + p[1] * z[:, h - 1::-1]
        second = (1 - p[2]) * z[:, h:] + p[2] * z[:, : h - 1 : -1]
        out = np.concatenate([first, second], axis=1).reshape(out.shape)
    return out


def _build_mt(logits):
    """M^T [1024, 1024] fp32: row j = transform(e_j), so MT[j, i] = M[i, j]."""
    eye = np.eye(SIZE, dtype=np.float64)
    mt = _transform64(eye, logits)
    return np.ascontiguousarray(mt.astype(np.float32))


def _build_bass():
    nc = bacc.Bacc("TRN2", target_bir_lowering=False, debug=False)
    xt = nc.dram_tensor("xt", [SIZE, ROWS_PER_CORE], BF16, kind="ExternalInput").ap()
    mt = nc.dram_tensor("mt", [SIZE, SIZE], BF16, kind="ExternalInput").ap()
    outt = nc.dram_tensor(
        "outt", [SIZE, ROWS_PER_CORE], BF16, kind="ExternalOutput"
    ).ap()

    with tile.TileContext(nc) as tc, ExitStack() as ctx:
        const = ctx.enter_context(tc.tile_pool(name="const", bufs=1))
        x0pool = ctx.enter_context(tc.tile_pool(name="x0", bufs=1))
        xpool = ctx.enter_context(tc.tile_pool(name="xin", bufs=3))
        opool = ctx.enter_context(tc.tile_pool(name="osb", bufs=3))
        pso = ctx.enter_context(tc.tile_pool(name="pso", bufs=4, space="PSUM"))

        # Group 0 loads chunk-granular, interleaved with the matching M^T
        # chunk, so MM(i=0, j) only waits on ~2 small transfers instead of
        # the whole 3 MiB of front matter (trims ~10us off the head).
        mts = []
        x0 = []
        for j in range(N_CHUNK):
            xc = x0pool.tile([P, GW], BF16, tag=f"x0{j}")
            nc.sync.dma_start(xc[:], xt[j * P : (j + 1) * P, 0:GW])
            x0.append(xc)
            t = const.tile([P, SIZE], BF16, tag=f"mt{j}")
            nc.sync.dma_start(t[:], mt[j * P : (j + 1) * P, :])
            mts.append(t)

        for g in range(N_GROUPS):
            r0 = g * GW
            last = g == N_GROUPS - 1
            if g > 0:
                xin = xpool.tile([P, N_CHUNK * GW], BF16, tag="xin")
                nc.sync.dma_start(
                    xin[:].rearrange("p (c r) -> p c r", r=GW),
                    xt[:, r0 : r0 + GW].rearrange("(c p) r -> p c r", p=P),
                )
            osb = opool.tile([P, N_CHUNK * GW], BF16, tag="osb")

            for i in range(N_CHUNK):
                po = pso.tile([P, GW], F32, tag="po")
                for j in range(N_CHUNK):
                    rhs = x0[j][:] if g == 0 else xin[:, j * GW : (j + 1) * GW]
                    nc.tensor.matmul(
                        po[:],
                        mts[j][:, i * P : (i + 1) * P],
                        rhs,
                        start=(j == 0),
                        stop=(j == N_CHUNK - 1),
                    )
                # Alternate PSUM->SBUF (fp32->bf16) copies across engines.
                dst = osb[:, i * GW : (i + 1) * GW]
                if i % 2 == 0:
                    nc.vector.tensor_copy(dst, po[:])
                else:
                    nc.scalar.copy(dst, po[:])
            nc.sync.dma_start(
                outt[:, r0 : r0 + GW].rearrange("(c p) r -> p c r", p=P),
                osb[:].rearrange("p (c r) -> p c r", r=GW),
            )

    nc.compile()
    return nc


def _get_nc():
    key = MATMUL_MODE
    if key not in _NC_CACHE:
        _NC_CACHE[key] = _build_bass()
    return _NC_CACHE[key]


def kernel(x, logits):
    x = np.asarray(x)
    logits = np.asarray(logits)
    assert x.shape == (BATCH, SIZE)

    mt = _build_mt(logits).astype(NP_BF16)
    nc = _get_nc()

    in_maps = []
    for i in range(N_CORES):
        xc = x[i * ROWS_PER_CORE : (i + 1) * ROWS_PER_CORE]
        xtc = np.ascontiguousarray(xc.T.astype(NP_BF16))
        in_maps.append({"xt": xtc, "mt": mt})

    kwargs = dict(TRACE_KWARGS)
    if TRACE:
        kwargs.setdefault("trace", True)
        kwargs.setdefault("trace_cores", [0])
    res = run_bass_kernel_spmd(nc, in_maps, core_ids=list(range(N_CORES)), **kwargs)
    global LAST_RESULTS
    LAST_RESULTS = res
    return np.concatenate(
        [res.results[i]["outt"].T.astype(np.float32) for i in range(N_CORES)], axis=0
    )


# revision 5
# speedup vs baseline: 1.1897x; 1.1897x over previous
"""TRN2 Bass kernel for nn_BlockPermProduct.

The reference applies 9 probabilistic block-permutation mixing steps to each
row of x [65536, 1024]. Every step is linear in x, so the whole transform is
``out = x @ M^T`` for a 1024x1024 matrix M built on the host (float64) from
the tiny (9, 3) logits. Structure analysis (block sparsity, hierarchical
ranks, Monarch tests) shows M sits exactly at the low-rank break-even point
at every scale, so a dense matmul is the right device algorithm; the wins
come from engineering it to the PE streaming floor:

  - bf16 end to end: x is pre-transposed AND cast to bf16 on the host, so
    the device does ZERO transposes (the old kernel burned a third of PE
    time on them); HBM traffic halves; FWL makes weight loads 2x faster.
  - out^T formulation: psum[i, r] = sum_j MT[j_block, i_block]^T @ xT[j, r]
    with the 64 constant 128x128 MT sub-blocks as stationary weights and
    xT chunks as N=512 moving operands. Weight loads are data-independent
    and hide under the 512-cycle streams.
  - i-outer / j-inner loop: each output chunk's PSUM bank drains (Vector
    and Scalar engines alternate on the fp32->bf16 copies) while the next
    chunk's matmuls stream -> no PE bubbles.
  - per-group 1 MiB DMAs (in and out), triple-buffered.

Host un-transposes and casts the bf16 out^T back to fp32. Measured rel err
~1e-3 (tolerance 2e-2). Sharding: pure data parallel over batch, 8 cores.
"""

import numpy as np
import ml_dtypes
from contextlib import ExitStack

import concourse.bass as bass
import concourse.bacc as bacc
import concourse.mybir as mybir
import concourse.tile as tile
from concourse.bass_utils import run_bass_kernel_spmd

BATCH = 65536
SIZE = 1024
N_CORES = 8
ROWS_PER_CORE = BATCH // N_CORES  # 8192
P = 128
N_CHUNK = SIZE // P  # 8
GW = 512  # rows ("r" columns of xT) per group
N_GROUPS = ROWS_PER_CORE // GW  # 16
HALF = 512

F32 = mybir.dt.float32
BF16 = mybir.dt.bfloat16
NP_BF16 = np.dtype(ml_dtypes.bfloat16)

MATMUL_MODE = "bf16_xt"

TRACE = False
TRACE_KWARGS = {}
LAST_RESULTS = None

_NC_CACHE = {}


def _transform64(y, logits):
    """Float64 port of the reference transform, applied to rows of y."""
    m = 10
    sizes = [SIZE >> i for i in range(m - 1)][::-1]  # [4, 8, ..., 1024]
    out = y
    for i in range(m - 2, -1, -1):
        n = sizes[i]
        p = 1.0 / (1.0 + np.exp(-logits[i].astype(np.float64)))
        z = out.reshape(-1, n)
        sep = z.reshape(-1, n // 2, 2).transpose(0, 2, 1).reshape(-1, n)
        z = (1 - p[0]) * z + p[0] * sep
        h = n // 2
        first = (1 - p[1]) * z[:, :h] + p[1] * z[:, h - 1::-1]
        second = (1 - p[2]) * z[:, h:] + p[2] * z[:, : h - 1 : -1]
        out = np.concatenate([first, second], axis=1).reshape(out.shape)
    return out


def _build_mt(logits):
    """M^T [1024, 1024] fp32: row j = transform(e_j), so MT[j, i] = M[i, j]."""
    eye = np.eye(SIZE, dtype=np.float64)
    mt = _transform64(eye, logits)
    return np.ascontiguousarray(mt.astype(np.float32))


def _build_bass():
    nc = bacc.Bacc("TRN2", target_bir_lowering=False, debug=False)
    xt = nc.dram_tensor("xt", [SIZE, ROWS_PER_CORE], BF16, kind="ExternalInput").ap()
    mt = nc.dram_tensor("mt", [SIZE, SIZE], BF16, kind="ExternalInput").ap()
    outt = nc.dram_tensor(
        "outt", [SIZE, ROWS_PER_CORE], BF16, kind="ExternalOutput"
    ).ap()

    with tile.TileContext(nc) as tc, ExitStack() as ctx:
        const = ctx.enter_context(tc.tile_pool(name="const", bufs=1))
        xpool = ctx.enter_context(tc.tile_pool(name="xin", bufs=3))
        opool = ctx.enter_context(tc.tile_pool(name="osb", bufs=3))
        pso = ctx.enter_context(tc.tile_pool(name="pso", bufs=4, space="PSUM"))

        # First group's xT load goes ahead of the M^T constants so the PE
        # isn't queued behind 2 MiB of weights.
        xin0 = xpool.tile([P, N_CHUNK * GW], BF16, tag="xin")
        nc.sync.dma_start(
            xin0[:].rearrange("p (c r) -> p c r", r=GW),
            xt[:, 0:GW].rearrange("(c p) r -> p c r", p=P),
        )

        # M^T resident in SBUF as 8 row-chunk tiles; lhsT slices are the
        # 128x128 sub-blocks mts[j][:, i*128:(i+1)*128].
        mts = []
        for j in range(N_CHUNK):
            t = const.tile([P, SIZE], BF16, tag=f"mt{j}")
            nc.sync.dma_start(t[:], mt[j * P : (j + 1) * P, :])
            mts.append(t)

        for g in range(N_GROUPS):
            r0 = g * GW
            if g == 0:
                xin = xin0
            else:
                xin = xpool.tile([P, N_CHUNK * GW], BF16, tag="xin")
                nc.sync.dma_start(
                    xin[:].rearrange("p (c r) -> p c r", r=GW),
                    xt[:, r0 : r0 + GW].rearrange("(c p) r -> p c r", p=P),
                )
            osb = opool.tile([P, N_CHUNK * GW], BF16, tag="osb")

            for i in range(N_CHUNK):
                po = pso.tile([P, GW], F32, tag="po")
                for j in range(N_CHUNK):
                    nc.tensor.matmul(
                        po[:],
                        mts[j][:, i * P : (i + 1) * P],
                        xin[:, j * GW : (j + 1) * GW],
                        start=(j == 0),
                        stop=(j == N_CHUNK - 1),
                    )
                # Alternate PSUM->SBUF (fp32->bf16) copies across engines.
                dst = osb[:, i * GW : (i + 1) * GW]
                if i % 2 == 0:
                    nc.vector.tensor_copy(dst, po[:])
                else:
                    nc.scalar.copy(dst, po[:])
            nc.sync.dma_start(
                outt[:, r0 : r0 + GW].rearrange("(c p) r -> p c r", p=P),
                osb[:].rearrange("p (c r) -> p c r", r=GW),
            )

    nc.compile()
    return nc


def _get_nc():
    key = MATMUL_MODE
    if key not in _NC_CACHE:
        _NC_CACHE[key] = _build_bass()
    return _NC_CACHE[key]


def kernel(x, logits):
    x = np.asarray(x)
    logits = np.asarray(logits)
    assert x.shape == (BATCH, SIZE)

    mt = _build_mt(logits).astype(NP_BF16)
    nc = _get_nc()

    in_maps = []
    for i in range(N_CORES):
        xc = x[i * ROWS_PER_CORE : (i + 1) * ROWS_PER_CORE]
        xtc = np.ascontiguousarray(xc.T.astype(NP_BF16))
        in_maps.append({"xt": xtc, "mt": mt})

    kwargs = dict(TRACE_KWARGS)
    if TRACE:
        kwargs.setdefault("trace", True)
        kwargs.setdefault("trace_cores", [0])
    res = run_bass_kernel_spmd(nc, in_maps, core_ids=list(range(N_CORES)), **kwargs)
    global LAST_RESULTS
    LAST_RESULTS = res
    return np.concatenate(
        [res.results[i]["outt"].T.astype(np.float32) for i in range(N_CORES)], axis=0
    )
